# revision 45
# baseline (speedup 1.0000x reference)
"""Trainium2 Bass kernel for the Air3D CNF ROM model (nn_Air3DCNFROM).

Model: out[b] = lx(x_b) + tau_b * u_b where
  lx = sqrt(x0^2 + x1^2) - 0.25
  u  = decoder MLP([fourier(x), alpha(tau)])  (106 -> 512 -> 512 -> 512 -> 1, tanh)
  alpha(tau) = linear interp at tau of a latent RK4 trajectory traj[101, 10].

Structure:
  * alpha0 is zeros and the pnode dynamics depend only on (a, t), so the RK4
    latent trajectory is IDENTICAL for every batch row: a [101, 10] table
    computed once on the host (float32, mirroring the reference arithmetic).
  * The per-sample decoder input row [fourier(x_b), alpha(tau_b), 1] (107
    values) is prepared host-side in fp32 and shipped bf16 (the appended ones
    row folds dec_b0 into the L1 matmul). The device runs the decoder MLP --
    99.99% of the model FLOPs -- as a pure bf16 matmul/tanh stream.
  * out = (lx - R + tau*b3) + tau * u_raw; the parenthesized term and tau are
    shipped in a psum-strip-aligned layout so the final combine is two DVE
    ops per 512-sample strip, no PE transposes.

Distribution: pure data parallel over 8 NeuronCores (batch 65536 -> 8 x 8192).

Schedule: skewed software pipeline over 512-sample tiles -- slot s emits
L2(s-3), L3(s-5), L4(s-6), L1(s) -- so the tensor engine sees one long
dense matmul stream at the 216 ns/matmul N=512 roofline (the HAM activity
window stays at the 2.4 GHz K=8/8 p-state; ~27 dummy warm-up matmuls
pre-boost it during the DMA-gated ramp) while the scalar engine's tanh
ACTs trail with ~2.3us/slot of slack. All matmuls are bf16 with fp32 PSUM
accumulation. DMAs: one dma_start runs on ONE engine at ~elem_bytes/180ns,
so ramp-critical tensors are split into partition-range chunks across the
sync/scalar/gpsimd trigger queues, with >=2KB contiguous elements.

Measured on 8xTRN2 (exec_time of slowest core): ~167 us; scale-relative
error 1.35e-3 (budget 2e-2).
"""
import numpy as np
import ml_dtypes

import concourse.bass as bass
import concourse.tile as tile
from concourse import bacc, mybir
from concourse.bass_utils import run_bass_kernel_spmd

F32 = mybir.dt.float32
BF16 = mybir.dt.bfloat16
AF = mybir.ActivationFunctionType
ALU = mybir.AluOpType
BF = ml_dtypes.bfloat16

N_CORES = 8
B = 65536
B_SHARD = B // N_CORES
NT = 512  # batch tile (psum free dim)
LAT = 10
STEPS = 101
DTAU = np.float32(0.01)
RADIUS = 0.25
N_FREQS = 16
MAX_FREQ = 10.0
K_IN = 107  # 96 fourier + 10 alpha + 1 ones (bias fold)


def _host_traj(pn_w0, pn_b0, pn_w1, pn_b1, pn_w2, pn_b2):
    """RK4 scan of the pnode ODE for a single zero-initialized latent,
    mirroring the reference's float32 arithmetic."""
    f32 = np.float32
    half_dtau = f32(0.5) * DTAU
    dtau6 = f32(0.01 / 6.0)
    two = f32(2.0)
    ts = np.linspace(0.0, 1.0, STEPS, dtype=np.float32)

    def f(t, a):
        inp = np.concatenate([a, np.full((1, 1), t, np.float32)], axis=1)
        h = np.tanh(inp @ pn_w0 + pn_b0)
        h = np.tanh(h @ pn_w1 + pn_b1)
        return h @ pn_w2 + pn_b2

    a = np.zeros((1, LAT), np.float32)
    traj = np.empty((STEPS, LAT), np.float32)
    traj[0] = a
    for i in range(STEPS - 1):
        t = ts[i]
        k1 = f(t, a)
        k2 = f(t + half_dtau, a + half_dtau * k1)
        k3 = f(t + half_dtau, a + half_dtau * k2)
        k4 = f(t + DTAU, a + DTAU * k3)
        a = a + dtau6 * (k1 + two * k2 + two * k3 + k4)
        traj[i + 1] = a
    return traj


def build_kernel(b_shard: int, batched_act: bool = True):
    """Build the single-core Bass program (SPMD across cores).

    Skewed pipeline: slot s emits L1(s), L2(s-1), L3(s-2), L4(s-3) so the PE
    instruction stream is dense (no phase bursts that outrun the scalar
    engine's ACT drain rate and stall PSUM recycling).

    batched_act=True (dec_b1 == dec_b2 == 0) fuses each layer's four
    [128,512] tanh blocks into two [128,1024] ACTs over psum bank pairs.
    """
    n_tiles = b_shard // NT
    q2 = b_shard // 1024  # strip col-blocks (tile pairs)

    nc = bacc.Bacc("TRN2", target_bir_lowering=False, debug=False,
                   detect_race_conditions=True)

    # ---- DRAM I/O
    # A single dma_start runs on ONE DMA engine at ~elem_bytes/180ns, so
    # every ramp-critical tensor is (a) laid out row-major in DRAM with >=2KB
    # contiguous runs per partition and (b) split into partition-range chunk
    # DMAs that fan out across engines. h0a is [K_IN, B_SHARD] row-major;
    # w1/w2 are m-pair-major [2, 128, 1024] so an L2/L3 output-column pair's
    # weights arrive as one 2KB-element stream just before its matmuls.
    d_h0a = nc.dram_tensor("h0a", [K_IN, b_shard], BF16,
                           kind="ExternalInput").ap()
    d_w0p = nc.dram_tensor("w0p", [K_IN, 512], BF16, kind="ExternalInput").ap()
    d_w1 = nc.dram_tensor("w1", [2, 128, 1024], BF16,
                          kind="ExternalInput").ap()
    d_w2 = nc.dram_tensor("w2", [2, 128, 1024], BF16,
                          kind="ExternalInput").ap()
    d_w3c = nc.dram_tensor("w3c", [128, 4], BF16, kind="ExternalInput").ap()
    d_taus = nc.dram_tensor("taus", [2, NT * q2], F32,
                            kind="ExternalInput").ap()
    d_lxs = nc.dram_tensor("lxs", [2, NT * q2], F32, kind="ExternalInput").ap()
    d_b1c = nc.dram_tensor("b1c", [128, 4], F32, kind="ExternalInput").ap()
    d_b2c = nc.dram_tensor("b2c", [128, 4], F32, kind="ExternalInput").ap()
    d_out = nc.dram_tensor("out", [2, NT * q2], F32, kind="ExternalOutput").ap()

    with tile.TileContext(nc) as tc:
        with tc.tile_pool(name="res", bufs=1) as res, \
             tc.tile_pool(name="tmp", bufs=2) as tmp, \
             tc.tile_pool(name="hp1", bufs=5) as hp1, \
             tc.tile_pool(name="hp2", bufs=4) as hp2, \
             tc.tile_pool(name="hp3", bufs=3) as hp3, \
             tc.tile_pool(name="ps", bufs=3, space="PSUM") as ps, \
             tc.tile_pool(name="psx", bufs=2, space="PSUM") as psx:

            # ---- resident tensors
            w0p_sb = res.tile([K_IN, 512], BF16, name="w0p_sb")
            # col layout 512*k + 128*m + mi; m-chunk DMA dst is a
            # [128, 4, 128] strided view
            w1_sb = res.tile([128, 2048], BF16, name="w1_sb")
            w2_sb = res.tile([128, 2048], BF16, name="w2_sb")
            w3_sb = res.tile([128, 4], BF16, name="w3_sb")
            b_sb = []
            for i, d_b in enumerate((d_b1c, d_b2c)):
                bt = res.tile([128, 4], F32, name=f"b{i}_sb")
                if not batched_act:
                    nc.sync.dma_start(bt[:], d_b)
                b_sb.append(bt)
            tau_sb = res.tile([33, NT * q2], F32, name="tau_sb")
            lx_sb = res.tile([33, NT * q2], F32, name="lx_sb")
            out_sb = res.tile([33, NT * q2], F32, name="out_sb")
            # all 16 tiles' decoder-input rows live in one resident buffer;
            # L1 matmuls read 512-col slices directly (no staging copy).
            h0a_all = res.tile([K_IN, b_shard], BF16, name="h0a_all")

            # ---- ramp-in DMAs: everything chunked by partition range so the
            # early transfers run on many engines concurrently. scalar's
            # queue carries only w0p (any later trigger there would sit
            # ahead of the tanh ACT stream and stall psum drain).
            def rsplit(p, n):
                b = [p * i // n for i in range(n + 1)]
                return list(zip(b[:-1], b[1:]))

            RC2 = [(0, 54), (54, 107)]
            with tc.high_priority():
                for r0, r1 in rsplit(K_IN, 4):
                    nc.sync.dma_start(h0a_all[r0:r1, 0:512], d_h0a[r0:r1, 0:512])
                for r0, r1 in rsplit(K_IN, 4):
                    nc.scalar.dma_start(w0p_sb[r0:r1, :], d_w0p[r0:r1, :])
                # h0a tiles 1-4 on gpsimd; w1 owns sync right after h0a[0]
                # (the L3 skew is deep enough that w2 can trail everything).
                for c0, c1 in ((512, 1024), (1024, 1536)):
                    for r0, r1 in RC2:
                        nc.gpsimd.dma_start(h0a_all[r0:r1, c0:c1],
                                            d_h0a[r0:r1, c0:c1])
                for mp in range(2):
                    for r0, r1 in rsplit(128, 4):
                        nc.sync.dma_start(w1_sb[r0:r1, bass.ts(mp, 1024)],
                                          d_w1[mp, r0:r1, :])
                for r0, r1 in RC2:
                    nc.gpsimd.dma_start(h0a_all[r0:r1, 1536:2560],
                                        d_h0a[r0:r1, 1536:2560])
                # NOTE: scalar-queue DMA entries occupy the queue for the
                # whole transfer (2-5us each) -- lending scalar any weight
                # chunk pushes the first ACT past the psum-ring deadline and
                # costs ~30us. Keep scalar for w0p only.
                for mp in range(2):
                    for i, (r0, r1) in enumerate(rsplit(128, 4)):
                        ea = nc.sync if i % 2 == 0 else nc.gpsimd
                        ea.dma_start(w2_sb[r0:r1, bass.ts(mp, 1024)],
                                     d_w2[mp, r0:r1, :])
                nc.sync.dma_start(w3_sb[:], d_w3c)
            nc.gpsimd.dma_start(tau_sb[0:33:32, :], d_taus)
            nc.gpsimd.dma_start(lx_sb[0:33:32, :], d_lxs)

            # ---- PE warm-up: 18 dummy matmuls fill the DMA-gated idle
            # window before the first real L1 (~8.4 -> ~14.4us), so the HAM
            # activity window is already boosted (K=8/8, 2.4 GHz) when real
            # data lands.
            dum = res.tile([128, 512], BF16, name="dum")
            nc.vector.memset(dum[:], 0.0)
            scrap = res.tile([128, 64], F32, name="scrap")
            for r in range(3):
                pd = ps.tile([128, 2 * NT], F32, tag="mm", name=f"dum_{r}")
                for i in range(9):
                    nc.tensor.matmul(pd[:, bass.ts(i % 2, NT)],
                                     dum[:, 0:128], dum[:],
                                     start=(i < 2), stop=(i >= 7))
                nc.vector.tensor_copy(scrap[:], pd[:, 0:64])

            h_tiles: dict = {}
            pu: dict = {}

            def emit_l1(t):
                hout = hp1.tile([128, 4 * NT], BF16, tag="h1", name=f"h1_{t}")
                h_tiles[(t, 1)] = hout
                rhs = h0a_all[:, bass.ts(t, NT)]
                for half in range(2):
                    p = ps.tile([128, 2 * NT], F32, tag="mm",
                                name=f"p_l1_{t}_{half}")
                    for m2 in range(2):
                        m = 2 * half + m2
                        nc.tensor.matmul(p[:, bass.ts(m2, NT)],
                                         w0p_sb[:, bass.ts(m, 128)], rhs,
                                         start=True, stop=True)
                    nc.scalar.activation(hout[:, bass.ts(half, 2 * NT)],
                                         p[:, 0:2 * NT], AF.Tanh)

            def emit_layer(t, layer):
                w_sb = w1_sb if layer == 2 else w2_sb
                hin = h_tiles.pop((t, layer - 1))
                hpool = hp2 if layer == 2 else hp3
                hout = hpool.tile([128, 4 * NT], BF16, tag=f"h{layer}",
                                  name=f"h{layer}_{t}")
                h_tiles[(t, layer)] = hout
                for half in range(2):
                    p = ps.tile([128, 2 * NT], F32, tag="mm",
                                name=f"p_l{layer}_{t}_{half}")
                    for m2 in range(2):
                        m = 2 * half + m2
                        for k in range(4):
                            off = 512 * m + 128 * k
                            nc.tensor.matmul(p[:, bass.ts(m2, NT)],
                                             w_sb[:, off:off + 128],
                                             hin[:, bass.ts(k, NT)],
                                             start=(k == 0), stop=(k == 3))
                    if batched_act:
                        nc.scalar.activation(hout[:, bass.ts(half, 2 * NT)],
                                             p[:, 0:2 * NT], AF.Tanh)
                    else:
                        bias = b_sb[layer - 2]
                        for m2 in range(2):
                            m = 2 * half + m2
                            nc.scalar.activation(
                                hout[:, bass.ts(m, NT)], p[:, bass.ts(m2, NT)],
                                AF.Tanh, bias=bias[:, m:m + 1])

            def emit_l4(t):
                # a tile PAIR's two [1, 512] u rows live in one [128, 512]
                # psum tile (1 bank) at partitions {0, 32}; bufs=2 so a new
                # pair's chain never waits on the old pair's combine reads.
                # PE psum writes only support base partitions {0, 32, 64}
                # (quadrant 3 is broken in HW). NOTE: a col-tiled
                # (tile_position) variant halves L4's span in theory but
                # slows EVERY matmul in the program by ~43ns (LDWEIGHTS
                # pull-ahead stops), a large net loss -- don't.
                p2, j = divmod(t, 2)
                if j == 0:
                    pu[p2] = psx.tile([128, NT], F32, tag="u",
                                      name=f"pu_{p2}")
                h3 = h_tiles.pop((t, 3))
                dst = pu[p2][32 * j:32 * j + 1, :]
                for k in range(4):
                    nc.tensor.matmul(dst, w3_sb[:, k:k + 1],
                                     h3[:, bass.ts(k, NT)],
                                     start=(k == 0), stop=(k == 3))

            def emit_combine(t):
                # out = lxs + taus * u for a PAIR of tiles in two [33, 512]
                # DVE ops: the pair's u strips sit at psum partitions {0, 32}
                # of one col block, matching the taus/lxs/out strip layout
                # (DVE time scales with free size, not partition count;
                # partitions 1..31 carry garbage and are never read back).
                if t % 2 == 0:
                    return
                p2 = t // 2
                cs = bass.ts(p2, NT)
                st = tmp.tile([33, NT], F32, tag="st", name=f"st_{t}")
                nc.vector.tensor_tensor(st[:], pu.pop(p2)[0:33, :],
                                        tau_sb[0:33, cs], op=ALU.mult)
                nc.vector.tensor_tensor(out_sb[0:33, cs], st[:],
                                        lx_sb[0:33, cs], op=ALU.add)
                nc.sync.dma_start(d_out[:, cs], out_sb[0:33:32, cs])

            # ---- skewed pipeline. L1 runs 3 slots ahead of L2 and 5 ahead
            # of L3 (h tiles buffer in SBUF) so the ramp's weight-chunk
            # deadlines trail the first matmul by several slots.
            # h0a tile-pair DMA issue slots: late enough that they don't
            # steal ramp bandwidth from w1/w2 (ramp slots are short, so a
            # fixed s%2 cadence would fire pairs 5-10 into the crunch), but
            # >=2 full slots (~17us) ahead of first use.
            pair_issue = {2: 5, 5: 7, 7: 9, 9: 11, 11: 13, 12: 15}
            for s in range(n_tiles + 6):
                t0 = pair_issue.get(s)
                if t0 is not None:
                    ea = nc.sync if s % 2 == 0 else nc.gpsimd
                    ce = min(t0 + 2, n_tiles) * NT
                    for r0, r1 in RC2:
                        ea.dma_start(h0a_all[r0:r1, t0 * NT:ce],
                                     d_h0a[r0:r1, t0 * NT:ce])
                # phase order L2, L3, L4, L1 spreads the slot's six psum-ring
                # acquisitions evenly and L4's psum-free matmuls cover the
                # ACT drain jitter right before L1's acquisitions (an
                # L1-burst at the slot seam ping-pongs against ring depth 3).
                # drain compression: tiles 14-15 run L3 (and 13-15 L4) one
                # slot earlier than the steady-state skew, shortening the
                # thin end-of-pipeline slots by one.
                l3_tiles = [t for t, sl in ((s - 5, 5), (s - 4, 4))
                            if (sl == 5 and 0 <= t <= 13)
                            or (sl == 4 and t in (14, 15))]
                l4_tiles = [t for t, sl in ((s - 6, 6), (s - 5, 5))
                            if (sl == 6 and 0 <= t <= 12)
                            or (sl == 5 and t in (13, 14, 15))]
                if 0 <= s - 3 < n_tiles:
                    emit_layer(s - 3, 2)
                for t in l3_tiles:
                    emit_layer(t, 3)
                for t in l4_tiles:
                    emit_l4(t)
                if s < n_tiles:
                    emit_l1(s)
                for t in l4_tiles:
                    emit_combine(t)

    nc.finalize()
    return nc


def _prepare_core_inputs(x, tau, dec_w0, dec_b0, dec_w1, dec_b1, dec_w2, dec_b2,
                         dec_w3, dec_b3, traj):
    """Host-side sharding + layout prep. Returns list of per-core in_maps."""
    n_tiles = B_SHARD // NT
    q2 = n_tiles // 2
    freqs = np.linspace(1.0, MAX_FREQ, N_FREQS, dtype=np.float32)
    ts = np.linspace(0.0, 1.0, STEPS, dtype=np.float32)

    # fourier features in native reference order: [B, 3, 32] -> [B, 96]
    proj = (2.0 * np.pi) * x[:, :, None].astype(np.float32) * freqs[None, None, :]
    phi = np.concatenate([np.sin(proj), np.cos(proj)], axis=-1).reshape(B, 96)
    # latent interpolation alpha(tau), float32 like the reference
    idx = np.clip(np.floor(tau / DTAU).astype(np.int32), 0, STEPS - 2)
    ratio = ((tau - ts[idx]) / DTAU)[:, None]
    alpha = traj[idx] + ratio * (traj[idx + 1] - traj[idx])
    feat = np.concatenate(
        [phi, alpha, np.ones((B, 1), np.float32)], axis=1)  # [B, 107]

    lxv = (np.sqrt(x[:, 0] ** 2 + x[:, 1] ** 2) - np.float32(RADIUS)
           + tau * np.float32(dec_b3.reshape(-1)[0]))

    w0p = np.concatenate([dec_w0, dec_b0.reshape(1, 512)], axis=0).astype(BF)

    def wlayout(w):
        # [mp, r, 512*(m%2) + 128*k + mi] <- w[128k + r, 128m + mi]
        a = w.reshape(4, 128, 4, 128).transpose(2, 1, 0, 3)  # [m, r, k, mi]
        a = a.reshape(2, 2, 128, 4, 128).transpose(0, 2, 1, 3, 4)
        return np.ascontiguousarray(a.reshape(2, 128, 1024)).astype(BF)

    w1b = wlayout(dec_w1)
    w2b = wlayout(dec_w2)
    w3c = np.ascontiguousarray(dec_w3.reshape(4, 128).T).astype(BF)
    b1c = np.ascontiguousarray(dec_b1.reshape(4, 128).T)
    b2c = np.ascontiguousarray(dec_b2.reshape(4, 128).T)

    in_maps = []
    for c in range(N_CORES):
        sl = slice(c * B_SHARD, (c + 1) * B_SHARD)
        h0a = np.ascontiguousarray(feat[sl].T).astype(BF)  # [K_IN, B_SHARD]
        # strip-interleaved layouts: tile t -> (row t % 2, cols 512*(t//2))
        taus = np.ascontiguousarray(
            tau[sl].reshape(q2, 2, NT).transpose(1, 0, 2).reshape(2, NT * q2))
        lxs = np.ascontiguousarray(
            lxv[sl].reshape(q2, 2, NT).transpose(1, 0, 2).reshape(2, NT * q2))
        in_maps.append({
            "h0a": h0a, "taus": taus, "lxs": lxs,
            "w0p": w0p, "w1": w1b, "w2": w2b, "w3c": w3c,
            "b1c": b1c, "b2c": b2c,
        })
    return in_maps


def run(inputs: dict, trace: bool = False):
    """Build, run on 8 cores, gather. Returns (out, BassKernelResults)."""
    traj = _host_traj(inputs["pn_w0"], inputs["pn_b0"], inputs["pn_w1"],
                      inputs["pn_b1"], inputs["pn_w2"], inputs["pn_b2"])
    batched = not (np.any(np.asarray(inputs["dec_b1"]))
                   or np.any(np.asarray(inputs["dec_b2"])))
    nc = build_kernel(B_SHARD, batched_act=batched)
    in_maps = _prepare_core_inputs(
        np.asarray(inputs["x"], np.float32), np.asarray(inputs["tau"], np.float32),
        np.asarray(inputs["dec_w0"], np.float32), np.asarray(inputs["dec_b0"], np.float32),
        np.asarray(inputs["dec_w1"], np.float32), np.asarray(inputs["dec_b1"], np.float32),
        np.asarray(inputs["dec_w2"], np.float32), np.asarray(inputs["dec_b2"], np.float32),
        np.asarray(inputs["dec_w3"], np.float32), np.asarray(inputs["dec_b3"], np.float32),
        traj)
    res = run_bass_kernel_spmd(nc, in_maps, list(range(N_CORES)), trace=trace)
    q2 = (B_SHARD // NT) // 2
    out = np.concatenate([
        res.results[c]["out"].reshape(2, q2, NT)
        .transpose(1, 0, 2).reshape(B_SHARD)
        for c in range(N_CORES)])
    return out, res


def kernel(**inputs) -> np.ndarray:
    out, _ = run(inputs, trace=False)
    return out


# revision 46
# speedup vs baseline: 1.0055x; 1.0055x over previous
"""Trainium2 Bass kernel for the Air3D CNF ROM model (nn_Air3DCNFROM).

Model: out[b] = lx(x_b) + tau_b * u_b where
  lx = sqrt(x0^2 + x1^2) - 0.25
  u  = decoder MLP([fourier(x), alpha(tau)])  (106 -> 512 -> 512 -> 512 -> 1, tanh)
  alpha(tau) = linear interp at tau of a latent RK4 trajectory traj[101, 10].

Structure:
  * alpha0 is zeros and the pnode dynamics depend only on (a, t), so the RK4
    latent trajectory is IDENTICAL for every batch row: a [101, 10] table
    computed once on the host (float32, mirroring the reference arithmetic).
  * The per-sample decoder input row [fourier(x_b), alpha(tau_b), 1] (107
    values) is prepared host-side in fp32 and shipped bf16 (the appended ones
    row folds dec_b0 into the L1 matmul). The device runs the decoder MLP --
    99.99% of the model FLOPs -- as a pure bf16 matmul/tanh stream.
  * out = (lx - R + tau*b3) + tau * u_raw; the parenthesized term and tau are
    shipped in a psum-strip-aligned layout so the final combine is two DVE
    ops per 512-sample strip, no PE transposes.

Distribution: pure data parallel over 8 NeuronCores (batch 65536 -> 8 x 8192).

Schedule: skewed software pipeline over 512-sample tiles -- slot s emits
L2(s-3), L3(s-5), L4(s-6), L1(s) -- so the tensor engine sees one long
dense matmul stream at the 216 ns/matmul N=512 roofline (the HAM activity
window stays at the 2.4 GHz K=8/8 p-state; ~27 dummy warm-up matmuls
pre-boost it during the DMA-gated ramp) while the scalar engine's tanh
ACTs trail with ~2.3us/slot of slack. All matmuls are bf16 with fp32 PSUM
accumulation. DMAs: one dma_start runs on ONE engine at ~elem_bytes/180ns,
so ramp-critical tensors are split into partition-range chunks across the
sync/scalar/gpsimd trigger queues, with >=2KB contiguous elements.

Measured on 8xTRN2 (exec_time of slowest core): ~167 us; scale-relative
error 1.35e-3 (budget 2e-2).
"""
import numpy as np
import ml_dtypes

import concourse.bass as bass
import concourse.tile as tile
from concourse import bacc, mybir
from concourse.bass_utils import run_bass_kernel_spmd

F32 = mybir.dt.float32
BF16 = mybir.dt.bfloat16
AF = mybir.ActivationFunctionType
ALU = mybir.AluOpType
BF = ml_dtypes.bfloat16

N_CORES = 8
B = 65536
B_SHARD = B // N_CORES
NT = 512  # batch tile (psum free dim)
LAT = 10
STEPS = 101
DTAU = np.float32(0.01)
RADIUS = 0.25
N_FREQS = 16
MAX_FREQ = 10.0
K_IN = 107  # 96 fourier + 10 alpha + 1 ones (bias fold)


def _host_traj(pn_w0, pn_b0, pn_w1, pn_b1, pn_w2, pn_b2):
    """RK4 scan of the pnode ODE for a single zero-initialized latent,
    mirroring the reference's float32 arithmetic."""
    f32 = np.float32
    half_dtau = f32(0.5) * DTAU
    dtau6 = f32(0.01 / 6.0)
    two = f32(2.0)
    ts = np.linspace(0.0, 1.0, STEPS, dtype=np.float32)

    def f(t, a):
        inp = np.concatenate([a, np.full((1, 1), t, np.float32)], axis=1)
        h = np.tanh(inp @ pn_w0 + pn_b0)
        h = np.tanh(h @ pn_w1 + pn_b1)
        return h @ pn_w2 + pn_b2

    a = np.zeros((1, LAT), np.float32)
    traj = np.empty((STEPS, LAT), np.float32)
    traj[0] = a
    for i in range(STEPS - 1):
        t = ts[i]
        k1 = f(t, a)
        k2 = f(t + half_dtau, a + half_dtau * k1)
        k3 = f(t + half_dtau, a + half_dtau * k2)
        k4 = f(t + DTAU, a + DTAU * k3)
        a = a + dtau6 * (k1 + two * k2 + two * k3 + k4)
        traj[i + 1] = a
    return traj


def build_kernel(b_shard: int, batched_act: bool = True):
    """Build the single-core Bass program (SPMD across cores).

    Skewed pipeline: slot s emits L1(s), L2(s-1), L3(s-2), L4(s-3) so the PE
    instruction stream is dense (no phase bursts that outrun the scalar
    engine's ACT drain rate and stall PSUM recycling).

    batched_act=True (dec_b1 == dec_b2 == 0) fuses each layer's four
    [128,512] tanh blocks into two [128,1024] ACTs over psum bank pairs.
    """
    n_tiles = b_shard // NT
    q2 = b_shard // 1024  # strip col-blocks (tile pairs)

    nc = bacc.Bacc("TRN2", target_bir_lowering=False, debug=False,
                   detect_race_conditions=True)

    # ---- DRAM I/O
    # A single dma_start runs on ONE DMA engine at ~elem_bytes/180ns, so
    # every ramp-critical tensor is (a) laid out row-major in DRAM with >=2KB
    # contiguous runs per partition and (b) split into partition-range chunk
    # DMAs that fan out across engines. h0a is [K_IN, B_SHARD] row-major;
    # w1/w2 are m-pair-major [2, 128, 1024] so an L2/L3 output-column pair's
    # weights arrive as one 2KB-element stream just before its matmuls.
    d_h0a = nc.dram_tensor("h0a", [K_IN, b_shard], BF16,
                           kind="ExternalInput").ap()
    d_w0p = nc.dram_tensor("w0p", [K_IN, 512], BF16, kind="ExternalInput").ap()
    d_w1 = nc.dram_tensor("w1", [2, 128, 1024], BF16,
                          kind="ExternalInput").ap()
    d_w2 = nc.dram_tensor("w2", [2, 128, 1024], BF16,
                          kind="ExternalInput").ap()
    d_w3c = nc.dram_tensor("w3c", [128, 4], BF16, kind="ExternalInput").ap()
    d_taus = nc.dram_tensor("taus", [2, NT * q2], F32,
                            kind="ExternalInput").ap()
    d_lxs = nc.dram_tensor("lxs", [2, NT * q2], F32, kind="ExternalInput").ap()
    d_b1c = nc.dram_tensor("b1c", [128, 4], F32, kind="ExternalInput").ap()
    d_b2c = nc.dram_tensor("b2c", [128, 4], F32, kind="ExternalInput").ap()
    d_out = nc.dram_tensor("out", [2, NT * q2], F32, kind="ExternalOutput").ap()

    with tile.TileContext(nc) as tc:
        with tc.tile_pool(name="res", bufs=1) as res, \
             tc.tile_pool(name="tmp", bufs=2) as tmp, \
             tc.tile_pool(name="hp1", bufs=5) as hp1, \
             tc.tile_pool(name="hp2", bufs=4) as hp2, \
             tc.tile_pool(name="hp3", bufs=3) as hp3, \
             tc.tile_pool(name="ps", bufs=3, space="PSUM") as ps, \
             tc.tile_pool(name="psx", bufs=2, space="PSUM") as psx:

            # ---- resident tensors
            w0p_sb = res.tile([K_IN, 512], BF16, name="w0p_sb")
            # col layout 512*k + 128*m + mi; m-chunk DMA dst is a
            # [128, 4, 128] strided view
            w1_sb = res.tile([128, 2048], BF16, name="w1_sb")
            w2_sb = res.tile([128, 2048], BF16, name="w2_sb")
            w3_sb = res.tile([128, 4], BF16, name="w3_sb")
            b_sb = []
            for i, d_b in enumerate((d_b1c, d_b2c)):
                bt = res.tile([128, 4], F32, name=f"b{i}_sb")
                if not batched_act:
                    nc.sync.dma_start(bt[:], d_b)
                b_sb.append(bt)
            tau_sb = res.tile([33, NT * q2], F32, name="tau_sb")
            lx_sb = res.tile([33, NT * q2], F32, name="lx_sb")
            out_sb = res.tile([33, NT * q2], F32, name="out_sb")
            # all 16 tiles' decoder-input rows live in one resident buffer;
            # L1 matmuls read 512-col slices directly (no staging copy).
            h0a_all = res.tile([K_IN, b_shard], BF16, name="h0a_all")

            # ---- ramp-in DMAs: everything chunked by partition range so the
            # early transfers run on many engines concurrently. scalar's
            # queue carries only w0p (any later trigger there would sit
            # ahead of the tanh ACT stream and stall psum drain).
            def rsplit(p, n):
                b = [p * i // n for i in range(n + 1)]
                return list(zip(b[:-1], b[1:]))

            RC2 = [(0, 54), (54, 107)]
            with tc.high_priority():
                for r0, r1 in rsplit(K_IN, 4):
                    nc.sync.dma_start(h0a_all[r0:r1, 0:512], d_h0a[r0:r1, 0:512])
                for r0, r1 in rsplit(K_IN, 4):
                    nc.scalar.dma_start(w0p_sb[r0:r1, :], d_w0p[r0:r1, :])
                # h0a tiles 1-4 on gpsimd; w1 owns sync right after h0a[0]
                # (the L3 skew is deep enough that w2 can trail everything).
                for c0, c1 in ((512, 1024), (1024, 1536)):
                    for r0, r1 in RC2:
                        nc.gpsimd.dma_start(h0a_all[r0:r1, c0:c1],
                                            d_h0a[r0:r1, c0:c1])
                for mp in range(2):
                    for r0, r1 in rsplit(128, 4):
                        nc.sync.dma_start(w1_sb[r0:r1, bass.ts(mp, 1024)],
                                          d_w1[mp, r0:r1, :])
                for r0, r1 in RC2:
                    nc.gpsimd.dma_start(h0a_all[r0:r1, 1536:2560],
                                        d_h0a[r0:r1, 1536:2560])
                # NOTE: scalar-queue DMA entries occupy the queue for the
                # whole transfer (2-5us each) -- lending scalar any weight
                # chunk pushes the first ACT past the psum-ring deadline and
                # costs ~30us. Keep scalar for w0p only.
                for mp in range(2):
                    for i, (r0, r1) in enumerate(rsplit(128, 4)):
                        ea = nc.sync if i % 2 == 0 else nc.gpsimd
                        ea.dma_start(w2_sb[r0:r1, bass.ts(mp, 1024)],
                                     d_w2[mp, r0:r1, :])
                nc.sync.dma_start(w3_sb[:], d_w3c)
            nc.gpsimd.dma_start(tau_sb[0:33:32, :], d_taus)
            nc.gpsimd.dma_start(lx_sb[0:33:32, :], d_lxs)

            # ---- PE warm-up: 18 dummy matmuls fill the DMA-gated idle
            # window before the first real L1 (~8.4 -> ~14.4us), so the HAM
            # activity window is already boosted (K=8/8, 2.4 GHz) when real
            # data lands.
            dum = res.tile([128, 512], BF16, name="dum")
            nc.vector.memset(dum[:], 0.0)
            scrap = res.tile([128, 64], F32, name="scrap")
            for r in range(3):
                pd = ps.tile([128, 2 * NT], F32, tag="mm", name=f"dum_{r}")
                for i in range(9):
                    nc.tensor.matmul(pd[:, bass.ts(i % 2, NT)],
                                     dum[:, 0:128], dum[:],
                                     start=(i < 2), stop=(i >= 7))
                nc.vector.tensor_copy(scrap[:], pd[:, 0:64])

            h_tiles: dict = {}
            pu: dict = {}

            def emit_l1(t):
                hout = hp1.tile([128, 4 * NT], BF16, tag="h1", name=f"h1_{t}")
                h_tiles[(t, 1)] = hout
                rhs = h0a_all[:, bass.ts(t, NT)]
                for half in range(2):
                    p = ps.tile([128, 2 * NT], F32, tag="mm",
                                name=f"p_l1_{t}_{half}")
                    for m2 in range(2):
                        m = 2 * half + m2
                        nc.tensor.matmul(p[:, bass.ts(m2, NT)],
                                         w0p_sb[:, bass.ts(m, 128)], rhs,
                                         start=True, stop=True)
                    nc.scalar.activation(hout[:, bass.ts(half, 2 * NT)],
                                         p[:, 0:2 * NT], AF.Tanh)

            def emit_layer(t, layer):
                w_sb = w1_sb if layer == 2 else w2_sb
                hin = h_tiles.pop((t, layer - 1))
                hpool = hp2 if layer == 2 else hp3
                hout = hpool.tile([128, 4 * NT], BF16, tag=f"h{layer}",
                                  name=f"h{layer}_{t}")
                h_tiles[(t, layer)] = hout
                for half in range(2):
                    p = ps.tile([128, 2 * NT], F32, tag="mm",
                                name=f"p_l{layer}_{t}_{half}")
                    for m2 in range(2):
                        m = 2 * half + m2
                        for k in range(4):
                            off = 512 * m + 128 * k
                            nc.tensor.matmul(p[:, bass.ts(m2, NT)],
                                             w_sb[:, off:off + 128],
                                             hin[:, bass.ts(k, NT)],
                                             start=(k == 0), stop=(k == 3))
                    if batched_act:
                        nc.scalar.activation(hout[:, bass.ts(half, 2 * NT)],
                                             p[:, 0:2 * NT], AF.Tanh)
                    else:
                        bias = b_sb[layer - 2]
                        for m2 in range(2):
                            m = 2 * half + m2
                            nc.scalar.activation(
                                hout[:, bass.ts(m, NT)], p[:, bass.ts(m2, NT)],
                                AF.Tanh, bias=bias[:, m:m + 1])

            def emit_l4(t):
                # a tile PAIR's two [1, 512] u rows live in one [128, 512]
                # psum tile (1 bank) at partitions {0, 32}; bufs=2 so a new
                # pair's chain never waits on the old pair's combine reads.
                # PE psum writes only support base partitions {0, 32, 64}
                # (quadrant 3 is broken in HW). NOTE: a col-tiled
                # (tile_position) variant halves L4's span in theory but
                # slows EVERY matmul in the program by ~43ns (LDWEIGHTS
                # pull-ahead stops), a large net loss -- don't.
                p2, j = divmod(t, 2)
                if j == 0:
                    pu[p2] = psx.tile([128, NT], F32, tag="u",
                                      name=f"pu_{p2}")
                h3 = h_tiles.pop((t, 3))
                dst = pu[p2][32 * j:32 * j + 1, :]
                for k in range(4):
                    nc.tensor.matmul(dst, w3_sb[:, k:k + 1],
                                     h3[:, bass.ts(k, NT)],
                                     start=(k == 0), stop=(k == 3))

            def emit_combine(t):
                # out = lxs + taus * u for a PAIR of tiles in two [33, 512]
                # DVE ops: the pair's u strips sit at psum partitions {0, 32}
                # of one col block, matching the taus/lxs/out strip layout
                # (DVE time scales with free size, not partition count;
                # partitions 1..31 carry garbage and are never read back).
                if t % 2 == 0:
                    return
                p2 = t // 2
                cs = bass.ts(p2, NT)
                st = tmp.tile([33, NT], F32, tag="st", name=f"st_{t}")
                nc.vector.tensor_tensor(st[:], pu.pop(p2)[0:33, :],
                                        tau_sb[0:33, cs], op=ALU.mult)
                nc.vector.tensor_tensor(out_sb[0:33, cs], st[:],
                                        lx_sb[0:33, cs], op=ALU.add)
                nc.sync.dma_start(d_out[:, cs], out_sb[0:33:32, cs])

            # ---- skewed pipeline. L1 runs 3 slots ahead of L2 and 5 ahead
            # of L3 (h tiles buffer in SBUF) so the ramp's weight-chunk
            # deadlines trail the first matmul by several slots.
            # h0a tile-pair DMA issue slots: late enough that they don't
            # steal ramp bandwidth from w1/w2 (ramp slots are short, so a
            # fixed s%2 cadence would fire pairs 5-10 into the crunch), but
            # >=2 full slots (~17us) ahead of first use.
            pair_issue = {2: 5, 5: 7, 7: 9, 9: 11, 11: 13, 12: 15}
            for s in range(n_tiles + 6):
                t0 = pair_issue.get(s)
                if t0 is not None:
                    ea = nc.sync if s % 2 == 0 else nc.gpsimd
                    ce = min(t0 + 2, n_tiles) * NT
                    for r0, r1 in RC2:
                        ea.dma_start(h0a_all[r0:r1, t0 * NT:ce],
                                     d_h0a[r0:r1, t0 * NT:ce])
                # phase order L2, L3, L4, L1 spreads the slot's six psum-ring
                # acquisitions evenly and L4's psum-free matmuls cover the
                # ACT drain jitter right before L1's acquisitions (an
                # L1-burst at the slot seam ping-pongs against ring depth 3).
                # drain compression: the last tiles run L2/L3/L4 one or two
                # slots earlier than the steady-state skew (their deps are
                # ready; the deep skew only matters during the DMA ramp),
                # ending the pipeline two slots sooner.
                l2_tiles = [t for t in (s - 3, s - 2)
                            if (t == s - 3 and 0 <= t <= 13)
                            or (t == s - 2 and t in (14, 15))]
                l3_tiles = [t for t in range(n_tiles)
                            if s == (t + 5 if t <= 12 else
                                     (t + 4 if t <= 14 else 18))]
                l4_tiles = [t for t in range(n_tiles)
                            if s == (t + 6 if t <= 12 else
                                     (18 if t == 13 else 19))]
                for t in l2_tiles:
                    emit_layer(t, 2)
                for t in l3_tiles:
                    emit_layer(t, 3)
                for t in l4_tiles:
                    emit_l4(t)
                if s < n_tiles:
                    emit_l1(s)
                for t in l4_tiles:
                    emit_combine(t)

    nc.finalize()
    return nc


def _prepare_core_inputs(x, tau, dec_w0, dec_b0, dec_w1, dec_b1, dec_w2, dec_b2,
                         dec_w3, dec_b3, traj):
    """Host-side sharding + layout prep. Returns list of per-core in_maps."""
    n_tiles = B_SHARD // NT
    q2 = n_tiles // 2
    freqs = np.linspace(1.0, MAX_FREQ, N_FREQS, dtype=np.float32)
    ts = np.linspace(0.0, 1.0, STEPS, dtype=np.float32)

    # fourier features in native reference order: [B, 3, 32] -> [B, 96]
    proj = (2.0 * np.pi) * x[:, :, None].astype(np.float32) * freqs[None, None, :]
    phi = np.concatenate([np.sin(proj), np.cos(proj)], axis=-1).reshape(B, 96)
    # latent interpolation alpha(tau), float32 like the reference
    idx = np.clip(np.floor(tau / DTAU).astype(np.int32), 0, STEPS - 2)
    ratio = ((tau - ts[idx]) / DTAU)[:, None]
    alpha = traj[idx] + ratio * (traj[idx + 1] - traj[idx])
    feat = np.concatenate(
        [phi, alpha, np.ones((B, 1), np.float32)], axis=1)  # [B, 107]

    lxv = (np.sqrt(x[:, 0] ** 2 + x[:, 1] ** 2) - np.float32(RADIUS)
           + tau * np.float32(dec_b3.reshape(-1)[0]))

    w0p = np.concatenate([dec_w0, dec_b0.reshape(1, 512)], axis=0).astype(BF)

    def wlayout(w):
        # [mp, r, 512*(m%2) + 128*k + mi] <- w[128k + r, 128m + mi]
        a = w.reshape(4, 128, 4, 128).transpose(2, 1, 0, 3)  # [m, r, k, mi]
        a = a.reshape(2, 2, 128, 4, 128).transpose(0, 2, 1, 3, 4)
        return np.ascontiguousarray(a.reshape(2, 128, 1024)).astype(BF)

    w1b = wlayout(dec_w1)
    w2b = wlayout(dec_w2)
    w3c = np.ascontiguousarray(dec_w3.reshape(4, 128).T).astype(BF)
    b1c = np.ascontiguousarray(dec_b1.reshape(4, 128).T)
    b2c = np.ascontiguousarray(dec_b2.reshape(4, 128).T)

    in_maps = []
    for c in range(N_CORES):
        sl = slice(c * B_SHARD, (c + 1) * B_SHARD)
        h0a = np.ascontiguousarray(feat[sl].T).astype(BF)  # [K_IN, B_SHARD]
        # strip-interleaved layouts: tile t -> (row t % 2, cols 512*(t//2))
        taus = np.ascontiguousarray(
            tau[sl].reshape(q2, 2, NT).transpose(1, 0, 2).reshape(2, NT * q2))
        lxs = np.ascontiguousarray(
            lxv[sl].reshape(q2, 2, NT).transpose(1, 0, 2).reshape(2, NT * q2))
        in_maps.append({
            "h0a": h0a, "taus": taus, "lxs": lxs,
            "w0p": w0p, "w1": w1b, "w2": w2b, "w3c": w3c,
            "b1c": b1c, "b2c": b2c,
        })
    return in_maps


def run(inputs: dict, trace: bool = False):
    """Build, run on 8 cores, gather. Returns (out, BassKernelResults)."""
    traj = _host_traj(inputs["pn_w0"], inputs["pn_b0"], inputs["pn_w1"],
                      inputs["pn_b1"], inputs["pn_w2"], inputs["pn_b2"])
    batched = not (np.any(np.asarray(inputs["dec_b1"]))
                   or np.any(np.asarray(inputs["dec_b2"])))
    nc = build_kernel(B_SHARD, batched_act=batched)
    in_maps = _prepare_core_inputs(
        np.asarray(inputs["x"], np.float32), np.asarray(inputs["tau"], np.float32),
        np.asarray(inputs["dec_w0"], np.float32), np.asarray(inputs["dec_b0"], np.float32),
        np.asarray(inputs["dec_w1"], np.float32), np.asarray(inputs["dec_b1"], np.float32),
        np.asarray(inputs["dec_w2"], np.float32), np.asarray(inputs["dec_b2"], np.float32),
        np.asarray(inputs["dec_w3"], np.float32), np.asarray(inputs["dec_b3"], np.float32),
        traj)
    res = run_bass_kernel_spmd(nc, in_maps, list(range(N_CORES)), trace=trace)
    q2 = (B_SHARD // NT) // 2
    out = np.concatenate([
        res.results[c]["out"].reshape(2, q2, NT)
        .transpose(1, 0, 2).reshape(B_SHARD)
        for c in range(N_CORES)])
    return out, res


def kernel(**inputs) -> np.ndarray:
    out, _ = run(inputs, trace=False)
    return out
